# revision 30
# baseline (speedup 1.0000x reference)
"""Cross-attention kernel for 8 TRN2 NeuronCores.

Reference computation (per batch b, c=1024 tokens, dim=1024):
    q = xf @ Wq.T ; k,v = cf @ Wkv.T split
    out = softmax(q @ k.T / 32) @ v

Algebraic restructure: scores = q @ k.T = x @ (Wq.T @ Wk) @ c.T, and
M = Wq.T @ Wk depends only on the weights, so it is precomputed on the
host.  This removes the k-projection matmul entirely — the device does
4 matmul phases per batch instead of 5 (t = x@M, v = c@Wv.T,
ST = t@c.T, out = softmax @ v).

Sharding: data-parallel over batch (16 batches -> 2 per core), SPMD on 8
cores, no collectives.  All activations enter the device pre-transposed
(host-side) so every matmul has its contraction dim on SBUF partitions:

    tT[o,i] = M.T @ xT            (lhsT=M[d,o],   rhs=xT[d,i])
    v[j,o]  = cT.T @ WvT          (lhsT=cT[d,j],  rhs=WvT[d,o])
    ST[j,i] = cT.T @ tT           (lhsT=cT[o,j],  rhs=tT[o,i])
    ET      = exp(ST/32)          (ACT, scale fused; no max-subtraction --
                                   logits are ~N(0,1), exp is fp32-safe)
    out'[i,o] = ET.T @ v          (lhsT=ET[j,i], rhs=v[j,o])

The ST (transposed-scores) formulation means the softmax matrix is never
transposed on device.  ET and the unnormalized out' stream back to the
host in fp16, and the softmax denominator + division happen there — this
keeps the device's matmul count at exactly 4 * 128 per batch with no
N=1 denominator matmuls riding the PE.

Schedule notes (from perfetto/NTFF analysis):
  - The PE issues one 512-row fp16 matmul every ~215 ns at full clock;
    1024 matmuls/core is the roofline (~220 us).  fp8 DoubleRow measures
    the SAME 215 ns/instruction on this hardware (the moving port is
    byte-bound), so fp8 hi/lo residual schemes lose; fp16 is optimal.
  - The framework preamble (~7 us) gates everything; warmup matmuls on
    garbage data bridge the HAM clock ramp (1.2 -> 2.4 GHz after ~3.4 us
    of sustained PE activity) until the first input DMAs land.
  - Batch 0's first phase-A half runs one ks-plane per pass across 8
    held PSUM banks so the first real matmul needs only m[0] + x[0]
    (~0.4 MB of DMA) instead of the full 3 MB operand set.
  - Batch 1's phase A is hoisted between phase C and phase D of batch 0
    to hide the exp-ACT latency on the last score tiles.
  - Input DMAs ride one queue in strict need order; output stores
    alternate between the sync and scalar HWDGE queues.
"""

import os
import sys

import numpy as np


def _ensure_paths():
    for p in ("/opt/trn_rl_repo", "/root/.axon_site/_ro/trn_rl_repo"):
        if os.path.isdir(p) and p not in sys.path:
            sys.path.append(p)


try:
    import concourse.bass  # noqa: F401
except ImportError:
    _ensure_paths()

try:
    # antenv initializes the axon PJRT runtime; without it the SPMD
    # result readback fails in a bare process.
    import antenv  # noqa: F401
except Exception:
    pass

import concourse.bass as bass  # noqa: E402
import concourse.tile as tile  # noqa: E402
from concourse import bacc, mybir  # noqa: E402
from concourse import bass_utils  # noqa: E402

B, C, HH, WW = 16, 1024, 32, 32
D = HH * WW  # 1024
NCORES = 8
BPC = B // NCORES  # 2 batches per core
P = 128
KS = D // P  # 8 contraction subtiles
NT = C // P  # 8 row tiles
NH = 512  # matmul moving free dim (one PSUM bank)
SCALE = float(D) ** -0.5

CDT = mybir.dt.float16  # on-device compute dtype
NPDT = np.float16

F32 = mybir.dt.float32

WARMUP_MMS = int(os.environ.get("KERNEL_WARMUP_MMS", "40"))


def _emit(tc, xT, cT, m, wv, out, eT):
    nc = tc.nc
    from contextlib import ExitStack

    ctx = ExitStack()
    with ctx:
        wpool = ctx.enter_context(tc.tile_pool(name="weights", bufs=1))
        iopool = ctx.enter_context(tc.tile_pool(name="io", bufs=2))
        actpool = ctx.enter_context(tc.tile_pool(name="acts", bufs=1))
        outpool = ctx.enter_context(tc.tile_pool(name="outs", bufs=3))
        psum = ctx.enter_context(tc.tile_pool(name="psum", bufs=8, space="PSUM"))

        # Pre-warm the PE during the startup DMA window: HAM un-throttles
        # (1.2 -> 2.4 GHz) only after ~3.4us of sustained PE activity, so a
        # burst of throwaway matmuls here means the real stream starts warm.
        if WARMUP_MMS:
            warm_in = wpool.tile([P, 128], CDT, tag="warm", name="warm_in")
            # GpSimd initializes earliest of all engines, so seeding the
            # warmup tile there lets the PE ramp start ~3us sooner than a
            # DVE memset would allow.
            nc.gpsimd.memset(warm_in[:], 0.0)
            warm_ps = psum.tile([P, 128], F32, tag="mm", name="warm_ps")
            for _ in range(WARMUP_MMS):
                nc.tensor.matmul(
                    warm_ps[:],
                    lhsT=warm_in[:],
                    rhs=warm_in[:],
                    start=True,
                    stop=True,
                )

        # Weights resident for the whole kernel; inputs for both batches
        # prefetched up front.  DMA issue order matches PE consumption
        # order (phase A needs m + batch-0 x first, then phase B needs
        # wv + batch-0 c, then the batch-1 inputs), split across the two
        # HWDGE queues so descriptor programming runs in parallel.
        w_sb = {
            name: wpool.tile([P, KS, D], CDT, tag=name, name=name)
            for name in ("m", "wv")
        }
        x_sbs = [
            iopool.tile([P, KS, C], CDT, tag="x", name="x_sb") for _ in range(BPC)
        ]
        c_sbs = [
            iopool.tile([P, KS, C], CDT, tag="c", name="c_sb") for _ in range(BPC)
        ]
        # One queue, strict need order: the PE's chain element ks unblocks
        # as each (m[ks], x[ks]) pair lands.  (A dual-queue split measures
        # worse: the queues race for the shared DMA engines and stretch
        # the per-descriptor completion times the chain is waiting on.)
        for ks in range(KS):
            nc.sync.dma_start(w_sb["m"][:, ks, :], m[ks])
            nc.sync.dma_start(x_sbs[0][:, ks, 0:NH], xT[0, ks, :, 0:NH])
        for ks in range(KS):
            nc.sync.dma_start(x_sbs[0][:, ks, NH:C], xT[0, ks, :, NH:C])
        for ks in range(KS):
            nc.sync.dma_start(w_sb["wv"][:, ks, :], wv[ks])
            nc.sync.dma_start(c_sbs[0][:, ks, :], cT[0, ks])
        for n in range(1, BPC):
            for ks in range(KS):
                nc.sync.dma_start(x_sbs[n][:, ks, :], xT[n, ks])
                nc.sync.dma_start(c_sbs[n][:, ks, :], cT[n, ks])

        def phase_a(n, tT_sb):
            x_sb = x_sbs[n]
            for ih in range(2):
                if n == 0 and ih == 0:
                    # Startup streaming: hold 8 PSUM banks (one per ot) and
                    # sweep the k-chain one ks-plane per pass, so the very
                    # first matmuls need only m[0] + x[0] (~0.4 MB) instead
                    # of the full m + x half (3 MB).  Each pass consumes the
                    # (m[ks], x[ks]) DMA pair that landed while the previous
                    # pass ran (a pass takes ~1.7us of PE time; the pair is
                    # ~0.4 MB, ~1.1us of DMA).
                    pss = [
                        psum.tile([P, NH], F32, tag="mm", name="ps_mm")
                        for _ in range(KS)
                    ]
                    for ks in range(KS):
                        for ot in range(KS):
                            nc.tensor.matmul(
                                pss[ot][:],
                                lhsT=w_sb["m"][:, ks, ot * P : (ot + 1) * P],
                                rhs=x_sb[:, ks, 0:NH],
                                start=(ks == 0),
                                stop=(ks == KS - 1),
                            )
                    for ot in range(KS):
                        nc.vector.tensor_copy(tT_sb[:, ot, 0:NH], pss[ot][:])
                    continue
                for ot in range(KS):
                    ps = psum.tile([P, NH], F32, tag="mm", name="ps_mm")
                    for ks in range(KS):
                        nc.tensor.matmul(
                            ps[:],
                            lhsT=w_sb["m"][:, ks, ot * P : (ot + 1) * P],
                            rhs=x_sb[:, ks, ih * NH : (ih + 1) * NH],
                            start=(ks == 0),
                            stop=(ks == KS - 1),
                        )
                    nc.vector.tensor_copy(
                        tT_sb[:, ot, ih * NH : (ih + 1) * NH], ps[:]
                    )

        tT_sbs = [
            actpool.tile([P, KS, C], CDT, tag=f"tT{n}", name=f"tT_sb{n}")
            for n in range(BPC)
        ]

        for n in range(BPC):
            x_sb = x_sbs[n]
            c_sb = c_sbs[n]
            tT_sb = tT_sbs[n]
            if n == 0:
                phase_a(0, tT_sb)

            # ---- phase B: v[j,o] = cT.T @ WvT ----
            # Depends only on DMA-landed inputs, so it fills the PE while
            # the DVE drains phase A's PSUM tiles.
            v_sb = actpool.tile([P, KS, D], CDT, tag="v", name="v_sb")
            for jt in range(NT):
                ps = [psum.tile([P, NH], F32, tag="mm", name="ps_mm") for _ in range(2)]
                for ks in range(KS):
                    for oh in range(2):
                        nc.tensor.matmul(
                            ps[oh][:],
                            lhsT=c_sb[:, ks, jt * P : (jt + 1) * P],
                            rhs=w_sb["wv"][:, ks, oh * NH : (oh + 1) * NH],
                            start=(ks == 0),
                            stop=(ks == KS - 1),
                        )
                for oh in range(2):
                    nc.vector.tensor_copy(
                        v_sb[:, jt, oh * NH : (oh + 1) * NH], ps[oh][:]
                    )

            # ---- phase C: ST[j,i] = cT.T @ tT -> ET = exp(ST/32) ----
            # ET streams to DRAM as it is produced; the softmax denominator
            # (row-sums of E) and the division are done on the host, which
            # removes the l-matmuls and the reciprocal from the device.
            eT_sb = actpool.tile([P, KS, C], CDT, tag="eT", name="eT_sb")
            for jt in range(NT):
                ps = [psum.tile([P, NH], F32, tag="mm", name="ps_mm") for _ in range(2)]
                for os_ in range(KS):
                    for ih in range(2):
                        nc.tensor.matmul(
                            ps[ih][:],
                            lhsT=c_sb[:, os_, jt * P : (jt + 1) * P],
                            rhs=tT_sb[:, os_, ih * NH : (ih + 1) * NH],
                            start=(os_ == 0),
                            stop=(os_ == KS - 1),
                        )
                for ih in range(2):
                    nc.scalar.activation(
                        eT_sb[:, jt, ih * NH : (ih + 1) * NH],
                        ps[ih][:],
                        mybir.ActivationFunctionType.Exp,
                        scale=SCALE,
                    )
                # eT stores stay off the scalar queue: a DMA_DIRECT2D there
                # would serialize with the exp ACTIVATEs and delay the last
                # score tile that phase D is waiting on.
                nc.sync.dma_start(eT[n, jt], eT_sb[:, jt, :])

            # ---- next batch's phase A: independent work that hides the
            # ---- tail ACT latency of phase C before phase D consumes ET.
            if n + 1 < BPC:
                phase_a(n + 1, tT_sbs[n + 1])

            # ---- phase D: out'[i,o] = ET.T @ v (unnormalized) ----
            for it in range(NT):
                o_sb = outpool.tile([P, D], CDT, tag="o", name="o_sb")
                ps = [psum.tile([P, NH], F32, tag="mm", name="ps_mm") for _ in range(2)]
                for js in range(NT):
                    lhsT = eT_sb[:, js, it * P : (it + 1) * P]
                    for oh in range(2):
                        nc.tensor.matmul(
                            ps[oh][:],
                            lhsT=lhsT,
                            rhs=v_sb[:, js, oh * NH : (oh + 1) * NH],
                            start=(js == 0),
                            stop=(js == NT - 1),
                        )
                last = n == BPC - 1 and it == NT - 1
                for oh in range(2):
                    if last and oh == 1:
                        # Final tile: drain the second PSUM half on the
                        # (idle) scalar engine in parallel with the DVE so
                        # the tail store chain is ~0.7us shorter.
                        nc.scalar.activation(
                            o_sb[:, oh * NH : (oh + 1) * NH],
                            ps[oh][:],
                            mybir.ActivationFunctionType.Copy,
                        )
                    else:
                        nc.vector.tensor_copy(
                            o_sb[:, oh * NH : (oh + 1) * NH], ps[oh][:]
                        )
                    eng = nc.sync if oh == 0 else nc.scalar
                    eng.dma_start(
                        out[n, it, :, oh * NH : (oh + 1) * NH],
                        o_sb[:, oh * NH : (oh + 1) * NH],
                    )


_NC_CACHE = {}


def _build():
    if "nc" in _NC_CACHE:
        return _NC_CACHE["nc"]
    nc = bacc.Bacc("TRN2", target_bir_lowering=False, debug=False)
    xT = nc.dram_tensor("xT", [BPC, KS, P, C], CDT, kind="ExternalInput").ap()
    cT = nc.dram_tensor("cT", [BPC, KS, P, C], CDT, kind="ExternalInput").ap()
    m = nc.dram_tensor("m", [KS, P, D], CDT, kind="ExternalInput").ap()
    wv = nc.dram_tensor("wv", [KS, P, D], CDT, kind="ExternalInput").ap()
    out = nc.dram_tensor("out", [BPC, NT, P, D], CDT, kind="ExternalOutput").ap()
    eT = nc.dram_tensor("eT", [BPC, NT, P, C], CDT, kind="ExternalOutput").ap()
    with tile.TileContext(nc) as tc:
        _emit(tc, xT, cT, m, wv, out, eT)
    nc.compile()
    _NC_CACHE["nc"] = nc
    return nc


def kernel(**inputs) -> np.ndarray:
    x = np.asarray(inputs["x"], dtype=np.float32).reshape(B, C, D)
    cond = np.asarray(inputs["cond_img"], dtype=np.float32).reshape(B, C, D)
    Wq = np.asarray(inputs["Wq"], dtype=np.float32)
    Wkv = np.asarray(inputs["Wkv"], dtype=np.float32)

    # Constant-fold the q/k projections: scores = x @ (Wq.T @ Wk) @ c.T.
    M = (Wq.T @ Wkv[:D]).astype(NPDT)  # (D_in, D_in), contraction dim first

    # Pre-transpose on host so the contraction dim lands on partitions.
    xT = np.ascontiguousarray(x.transpose(0, 2, 1)).astype(NPDT)  # (B, D, C)
    cT = np.ascontiguousarray(cond.transpose(0, 2, 1)).astype(NPDT)
    wvT = np.ascontiguousarray(Wkv[D:].T).astype(NPDT)

    xT = xT.reshape(NCORES, BPC, KS, P, C)
    cT = cT.reshape(NCORES, BPC, KS, P, C)
    m = M.reshape(KS, P, D)
    wv = wvT.reshape(KS, P, D)

    in_maps = [
        {"xT": xT[i], "cT": cT[i], "m": m, "wv": wv}
        for i in range(NCORES)
    ]

    nc = _build()
    trace = bool(os.environ.get("KERNEL_TRACE"))
    res = bass_utils.run_bass_kernel_spmd(
        nc, in_maps, core_ids=list(range(NCORES)), trace=trace
    )
    if trace:
        _NC_CACHE["last_result"] = res

    outs = np.stack([np.asarray(res.results[i]["out"]) for i in range(NCORES)])
    eTs = np.stack([np.asarray(res.results[i]["eT"]) for i in range(NCORES)])
    # Softmax denominator + division on host: l[i] = sum_j E[j, i].
    outs = outs.reshape(B, C, D).astype(np.float32)
    l = eTs.reshape(B, C, C).astype(np.float32).sum(axis=1)  # (B, i)
    outs /= l[:, :, None]
    return outs.reshape(B, C, HH, WW)
